# revision 3
# baseline (speedup 1.0000x reference)
"""AIMv2 attention (B=4, S=2048, D=1024, H=16, d=64) on 8 TRN2 NeuronCores.

Sharding: core c = (batch b = c//2, head-group g = c%2 of 8 heads).
Each core computes its batch's attention for its 8 heads plus the
out-projection partial sum over its heads' rows of w_out; the host adds
the two partials per batch (no on-device collectives needed).

Per-core kernel (all matmuls in bf16, fp32 accumulation):
  X^T via DVE cast + XBAR DMA transposes; Q^T,K^T = Wq/k^T @ X^T so the
  score matmuls produce s_T[k, q] directly; softmax without max-subtraction
  (scores ~ N(0,1), exp is safe); denominator via ones-vector matmuls;
  ctx^T = V^T @ P^T lands in the exact lhsT layout the out-projection needs.
"""

import numpy as np

import concourse.bass as bass
import concourse.tile as tile
from concourse import bacc, mybir
from concourse.bass_utils import run_bass_kernel_spmd

P = 128
S = 2048          # sequence length
D = 1024          # model dim
DQ = 512          # per-core qkv width (8 heads x 64)
HD = 64           # head dim
NH = 8            # heads per core
NKT = D // P      # 8 contraction tiles over D
NST = S // P      # 16 tiles over S
QC = 1024         # q chunk for attention inner loop
SCALE = 1.0 / 8.0  # 1/sqrt(64)

F32 = mybir.dt.float32
BF16 = mybir.dt.bfloat16


def build_kernel(nc, out_ap, hs_ap, wqkv_ap, wout_ap):
    import contextlib

    ctx = contextlib.ExitStack()
    with tile.TileContext(nc) as tc:
        with ctx:
            _body(ctx, tc, nc, out_ap, hs_ap, wqkv_ap, wout_ap)


def _body(ctx, tc, nc, out_ap, hs_ap, wqkv_ap, wout_ap):
    Exp = mybir.ActivationFunctionType.Exp

    persist = ctx.enter_context(tc.tile_pool(name="persist", bufs=1))
    psum = ctx.enter_context(tc.tile_pool(name="psum", bufs=1, space="PSUM"))

    # ---- constants ----
    ones_col = persist.tile([P, 1], BF16, name="ones_col")
    nc.vector.memset(ones_col[:], 1.0)
    ones_row = persist.tile([1, HD], F32, name="ones_row")
    nc.vector.memset(ones_row[:], 1.0)

    # ---- load phase (staging pools released before attention pools open) --
    xt = [persist.tile([P, S], BF16, name=f"xt{dt}") for dt in range(NKT)]
    wqkv_bf = []
    wout_bf = []
    with tc.tile_pool(name="stage", bufs=3) as stage:
        for kt in range(NKT):
            wf = stage.tile([P, 3 * DQ], F32, tag="wstage", bufs=2)
            nc.sync.dma_start(wf[:], wqkv_ap[kt * P:(kt + 1) * P, :])
            wb = persist.tile([P, 3 * DQ], BF16, name=f"wqkv_bf{kt}")
            nc.vector.tensor_copy(wb[:], wf[:])
            wqkv_bf.append(wb)

        for i in range(DQ // P):
            wf = stage.tile([P, D], F32, tag="wostage", bufs=2)
            nc.sync.dma_start(wf[:], wout_ap[i * P:(i + 1) * P, :])
            wb = persist.tile([P, D], BF16, name=f"wout_bf{i}")
            nc.vector.tensor_copy(wb[:], wf[:])
            wout_bf.append(wb)

        # X^T: load X, cast bf16, XBAR-transpose into [D, S] layout
        for st in range(NST):
            xf = stage.tile([P, D], F32, tag="xstage", bufs=3)
            nc.sync.dma_start(xf[:], hs_ap[st * P:(st + 1) * P, :])
            xb = stage.tile([P, D], BF16, tag="xbf", bufs=3)
            nc.vector.tensor_copy(xb[:], xf[:])
            for dt in range(NKT):
                nc.sync.dma_start_transpose(
                    xt[dt][:, st * P:(st + 1) * P], xb[:, dt * P:(dt + 1) * P]
                )

    pt_pool = ctx.enter_context(tc.tile_pool(name="pt", bufs=4))
    small = ctx.enter_context(tc.tile_pool(name="small", bufs=4))
    outsb_pool = ctx.enter_context(tc.tile_pool(name="outsb", bufs=2))

    # ---- V projection: v[st] = (X @ Wv)[st*128:(st+1)*128, :] in bf16 ----
    v_sb = [persist.tile([P, DQ], BF16, name=f"v{st}") for st in range(NST)]
    for stq in range(NST // 2):
        ps = psum.tile([P, 2 * DQ], F32, tag="sc", bufs=2)
        for half in range(2):
            st = 2 * stq + half
            sl = slice(half * DQ, (half + 1) * DQ)
            for kt in range(NKT):
                nc.tensor.matmul(
                    ps[:, sl],
                    lhsT=xt[kt][:, st * P:(st + 1) * P],
                    rhs=wqkv_bf[kt][:, 2 * DQ:3 * DQ],
                    start=(kt == 0),
                    stop=(kt == NKT - 1),
                )
        for half in range(2):
            st = 2 * stq + half
            nc.vector.tensor_copy(
                v_sb[st][:], ps[:, half * DQ:(half + 1) * DQ]
            )

    # ---- Q^T / K^T projections, per head pair (interleaved with attention
    # emission order so PE projection work overlaps ACT exp work) ----
    qt = [persist.tile([P, S], BF16, name=f"qt{m}") for m in range(4)]
    kt_sb = [persist.tile([P, S], BF16, name=f"kt{m}") for m in range(4)]

    def project_pair(m):
        # rows 128m..128m+128 of Q^T and K^T (heads 2m, 2m+1)
        for which, dst in ((0, qt[m]), (DQ, kt_sb[m])):
            for nqq in range(2):
                ps = psum.tile([P, 2 * 512], F32, tag="sc", bufs=2)
                for half in range(2):
                    nq = 2 * nqq + half
                    sl = slice(half * 512, (half + 1) * 512)
                    for kt in range(NKT):
                        nc.tensor.matmul(
                            ps[:, sl],
                            lhsT=wqkv_bf[kt][:, which + m * P: which + (m + 1) * P],
                            rhs=xt[kt][:, nq * 512:(nq + 1) * 512],
                            start=(kt == 0),
                            stop=(kt == NKT - 1),
                        )
                nc.vector.tensor_copy(
                    dst[:, nqq * 1024:(nqq + 1) * 1024], ps[:]
                )

    # ---- attention ----
    ctxt = [persist.tile([P, S], BF16, name=f"ctxt{m}") for m in range(4)]

    def attend(hp, qc):
        """Heads (2hp, 2hp+1): even head on partitions 0-63, odd on 64-127."""
        q0 = qc * QC
        ctxps = psum.tile([P, QC], F32, tag="ctx", bufs=1)
        sumps = psum.tile([P, QC], F32, tag="sum", bufs=1)
        for kti in range(NST):
            psA = psum.tile([P, QC], F32, tag="sc", bufs=2)
            psB = psum.tile([P, QC], F32, tag="sc", bufs=2)
            for half in range(2):
                sl = slice(half * 512, (half + 1) * 512)
                qsl = slice(q0 + half * 512, q0 + (half + 1) * 512)
                # row-group packed pair: A uses partitions 0-63, B 64-127
                nc.tensor.matmul(
                    psA[:, sl],
                    lhsT=kt_sb[hp][0:HD, kti * P:(kti + 1) * P],
                    rhs=qt[hp][0:HD, qsl],
                    start=True, stop=True,
                )
                nc.tensor.matmul(
                    psB[:, sl],
                    lhsT=kt_sb[hp][HD:P, kti * P:(kti + 1) * P],
                    rhs=qt[hp][HD:P, qsl],
                    start=True, stop=True,
                )
            ptA = pt_pool.tile([P, QC], BF16, tag="pt", bufs=4)
            ptB = pt_pool.tile([P, QC], BF16, tag="pt", bufs=4)
            nc.scalar.activation(ptA[:], psA[:], Exp, scale=SCALE)
            nc.scalar.activation(ptB[:], psB[:], Exp, scale=SCALE)
            first = kti == 0
            last = kti == NST - 1
            for half in range(2):
                sl = slice(half * 512, (half + 1) * 512)
                # ctx^T accumulation, col-group packed A/B
                nc.tensor.matmul(
                    ctxps[0:HD, sl],
                    lhsT=v_sb[kti][:, (2 * hp) * HD:(2 * hp + 1) * HD],
                    rhs=ptA[:, sl],
                    start=first, stop=last,
                    tile_position=(0, 0),
                )
                nc.tensor.matmul(
                    ctxps[HD:P, sl],
                    lhsT=v_sb[kti][:, (2 * hp + 1) * HD:(2 * hp + 2) * HD],
                    rhs=ptB[:, sl],
                    start=first, stop=last,
                    tile_position=(0, HD),
                )
                # softmax denominators
                nc.tensor.matmul(
                    sumps[0:1, sl],
                    lhsT=ones_col[:],
                    rhs=ptA[:, sl],
                    start=first, stop=last,
                    tile_position=(0, 0),
                )
                nc.tensor.matmul(
                    sumps[HD:HD + 1, sl],
                    lhsT=ones_col[:],
                    rhs=ptB[:, sl],
                    start=first, stop=last,
                    tile_position=(0, HD),
                )
        # normalize: ctx^T[d, q] * (1/sum[q]) broadcast across partitions
        recipA = small.tile([1, QC], F32, tag="recip", bufs=4)
        recipB = small.tile([1, QC], F32, tag="recip", bufs=4)
        nc.vector.reciprocal(recipA[:], sumps[0:1, :])
        nc.vector.reciprocal(recipB[:], sumps[HD:HD + 1, :])
        bc = psum.tile([P, QC], F32, tag="sum", bufs=1)
        for half in range(2):
            sl = slice(half * 512, (half + 1) * 512)
            nc.tensor.matmul(
                bc[0:HD, sl], lhsT=ones_row[:], rhs=recipA[:, sl],
                start=True, stop=True, tile_position=(0, 0),
            )
            nc.tensor.matmul(
                bc[HD:P, sl], lhsT=ones_row[:], rhs=recipB[:, sl],
                start=True, stop=True, tile_position=(0, HD),
            )
        bcs = small.tile([P, QC], F32, tag="bcs", bufs=2)
        nc.vector.tensor_copy(bcs[:], bc[:])
        nc.vector.tensor_mul(ctxt[hp][:, q0:q0 + QC], ctxps[:], bcs[:])

    project_pair(0)
    for qc in range(2):
        for hp in range(4):
            # emit next pair's projection between attention blocks so the
            # scheduler can fill PE idle time while ACT chews on exps
            attend(hp, qc)
            if qc == 0 and hp < 3:
                project_pair(hp + 1)

    # ---- out projection: out[st] = ctx^T.T @ w_out (partial over our heads)
    for st in range(NST):
        ps = psum.tile([P, D], F32, tag="sc", bufs=2)
        for half in range(2):
            sl = slice(half * 512, (half + 1) * 512)
            for c in range(4):
                nc.tensor.matmul(
                    ps[:, sl],
                    lhsT=ctxt[c][:, st * P:(st + 1) * P],
                    rhs=wout_bf[c][:, sl],
                    start=(c == 0),
                    stop=(c == 3),
                )
        osb = outsb_pool.tile([P, D], F32, tag="osb", bufs=3)
        nc.vector.tensor_copy(osb[:], ps[:])
        nc.sync.dma_start(out_ap[st * P:(st + 1) * P, :], osb[:])


_CACHED = None


def _get_nc():
    global _CACHED
    if _CACHED is None:
        nc = bacc.Bacc(
            "TRN2", target_bir_lowering=False, debug=False, num_devices=8
        )
        hs = nc.dram_tensor("hs", [S, D], F32, kind="ExternalInput").ap()
        wqkv = nc.dram_tensor("wqkv", [D, 3 * DQ], F32, kind="ExternalInput").ap()
        wout = nc.dram_tensor("wout", [DQ, D], F32, kind="ExternalInput").ap()
        out = nc.dram_tensor("out", [S, D], F32, kind="ExternalOutput").ap()
        build_kernel(nc, out, hs, wqkv, wout)
        nc.compile()
        _CACHED = nc
    return _CACHED


def make_in_maps(hidden_states, w_qkv, w_out):
    in_maps = []
    for c in range(8):
        b, g = divmod(c, 2)
        cols = slice(g * DQ, (g + 1) * DQ)
        wq = w_qkv[:, 0 * D:1 * D][:, cols]
        wk = w_qkv[:, 1 * D:2 * D][:, cols]
        wv = w_qkv[:, 2 * D:3 * D][:, cols]
        in_maps.append({
            "hs": np.ascontiguousarray(hidden_states[b], dtype=np.float32),
            "wqkv": np.ascontiguousarray(
                np.concatenate([wq, wk, wv], axis=1), dtype=np.float32
            ),
            "wout": np.ascontiguousarray(
                w_out[g * DQ:(g + 1) * DQ, :], dtype=np.float32
            ),
        })
    return in_maps


def run(hidden_states, w_qkv, w_out, trace=False):
    nc = _get_nc()
    in_maps = make_in_maps(hidden_states, w_qkv, w_out)
    res = run_bass_kernel_spmd(nc, in_maps, core_ids=list(range(8)), trace=trace)
    out = np.empty((4, S, D), np.float32)
    for b in range(4):
        out[b] = res.results[2 * b]["out"] + res.results[2 * b + 1]["out"]
    return out, res


def kernel(hidden_states, w_qkv, w_out):
    out, _ = run(
        np.asarray(hidden_states), np.asarray(w_qkv), np.asarray(w_out)
    )
    return out


# revision 6
# speedup vs baseline: 1.0286x; 1.0286x over previous
"""AIMv2 attention (B=4, S=2048, D=1024, H=16, d=64) on 8 TRN2 NeuronCores.

Sharding: core c = (batch b = c//2, head-group g = c%2 of 8 heads).
Each core computes its batch's attention for its 8 heads plus the
out-projection partial sum over its heads' rows of w_out; the host adds
the two partials per batch (no on-device collectives needed).

Per-core kernel (all matmuls in bf16, fp32 accumulation):
  X^T via DVE cast + batched XBAR DMA transposes; Q^T,K^T = Wq/k^T @ X^T
  so the score matmuls produce s_T[k, q] directly with heads packed in
  row-groups (even head partitions 0-63, odd 64-127 -> PE row tiling);
  softmax without max-subtraction (scores ~ N(0,1), exp never overflows);
  V carries a ones column so ctx' = [V|1]^T @ P^T yields both ctx^T and
  the softmax denominators in one accumulation; normalization via
  reciprocal_approx_fast + a K=1 broadcast matmul; ctx^T lands in the
  exact lhsT layout the out-projection needs.
"""

import numpy as np

import concourse.bass as bass
import concourse.tile as tile
from concourse import bacc, mybir
from concourse.bass_utils import run_bass_kernel_spmd

P = 128
S = 2048          # sequence length
D = 1024          # model dim
DQ = 512          # per-core qkv width (8 heads x 64)
HD = 64           # head dim
NH = 8            # heads per core
NKT = D // P      # 8 contraction tiles over D
NST = S // P      # 16 tiles over S
QC = 1024         # q chunk for attention inner loop
SCALE = 1.0 / 8.0  # 1/sqrt(64)

F32 = mybir.dt.float32
BF16 = mybir.dt.bfloat16


def build_kernel(nc, out_ap, hs_ap, wqkv_ap, wout_ap):
    import contextlib

    ctx = contextlib.ExitStack()
    with tile.TileContext(nc) as tc:
        with ctx:
            _body(ctx, tc, nc, out_ap, hs_ap, wqkv_ap, wout_ap)


def _body(ctx, tc, nc, out_ap, hs_ap, wqkv_ap, wout_ap):
    Exp = mybir.ActivationFunctionType.Exp

    persist = ctx.enter_context(tc.tile_pool(name="persist", bufs=1))
    psum = ctx.enter_context(tc.tile_pool(name="psum", bufs=1, space="PSUM"))

    ones_row = persist.tile([1, HD], F32, name="ones_row")
    nc.vector.memset(ones_row[:], 1.0)

    # ---- load phase (staging pools released before attention pools open) --
    # X^T lives as one 3D tile so each X row-tile transposes in ONE XBAR DMA.
    xt3 = persist.tile([P, NKT, S], BF16, name="xt3")
    wqkv_bf = []
    wout_bf = []
    with tc.tile_pool(name="stage", bufs=3) as stage:
        for kt in range(NKT):
            wf = stage.tile([P, 3 * DQ], F32, tag="wstage", bufs=2)
            nc.sync.dma_start(wf[:], wqkv_ap[kt * P:(kt + 1) * P, :])
            wb = persist.tile([P, 3 * DQ], BF16, name=f"wqkv_bf{kt}")
            nc.scalar.copy(wb[:], wf[:])
            wqkv_bf.append(wb)

        for i in range(DQ // P):
            wf = stage.tile([P, D], F32, tag="wostage", bufs=2)
            nc.sync.dma_start(wf[:], wout_ap[i * P:(i + 1) * P, :])
            wb = persist.tile([P, D], BF16, name=f"wout_bf{i}")
            nc.scalar.copy(wb[:], wf[:])
            wout_bf.append(wb)

        # X: load, cast bf16, batched XBAR transpose into [D, S] layout
        for st in range(NST):
            xf = stage.tile([P, D], F32, tag="xstage", bufs=3)
            nc.sync.dma_start(xf[:], hs_ap[st * P:(st + 1) * P, :])
            xb = stage.tile([P, D], BF16, tag="xbf", bufs=3)
            nc.vector.tensor_copy(xb[:], xf[:])
            nc.sync.dma_start_transpose(xt3[:, :, st * P:(st + 1) * P], xb[:])

    pt_pool = ctx.enter_context(tc.tile_pool(name="pt", bufs=4))
    small = ctx.enter_context(tc.tile_pool(name="small", bufs=4))
    outsb_pool = ctx.enter_context(tc.tile_pool(name="outsb", bufs=2))

    def xt(kt):
        return xt3[:, kt, :]

    # ---- V projection with ones column: vc[st][:, h, 0:64]=V_h, [...,64]=1
    vc = [persist.tile([P, NH, HD + 1], BF16, name=f"vc{st}") for st in range(NST)]
    for st in range(NST):
        nc.vector.memset(vc[st][:, :, HD:HD + 1], 1.0)
    for stq in range(NST // 2):
        ps = psum.tile([P, 2 * DQ], F32, tag="sc", bufs=2)
        for half in range(2):
            st = 2 * stq + half
            sl = slice(half * DQ, (half + 1) * DQ)
            for kt in range(NKT):
                nc.tensor.matmul(
                    ps[:, sl],
                    lhsT=xt(kt)[:, st * P:(st + 1) * P],
                    rhs=wqkv_bf[kt][:, 2 * DQ:3 * DQ],
                    start=(kt == 0),
                    stop=(kt == NKT - 1),
                )
        for half in range(2):
            st = 2 * stq + half
            for h in range(NH):
                nc.scalar.copy(
                    vc[st][:, h, 0:HD],
                    ps[:, half * DQ + h * HD: half * DQ + (h + 1) * HD],
                )

    # ---- Q^T / K^T projections, per head pair ----
    qt = [persist.tile([P, S], BF16, name=f"qt{m}") for m in range(4)]
    kt_sb = [persist.tile([P, S], BF16, name=f"kt{m}") for m in range(4)]

    def project_pair(m):
        # rows 128m..128m+128 of Q^T and K^T (heads 2m, 2m+1)
        for which, dst in ((0, qt[m]), (DQ, kt_sb[m])):
            for nqq in range(2):
                ps = psum.tile([P, 2 * 512], F32, tag="sc", bufs=2)
                for half in range(2):
                    nq = 2 * nqq + half
                    sl = slice(half * 512, (half + 1) * 512)
                    for kt in range(NKT):
                        nc.tensor.matmul(
                            ps[:, sl],
                            lhsT=wqkv_bf[kt][:, which + m * P: which + (m + 1) * P],
                            rhs=xt(kt)[:, nq * 512:(nq + 1) * 512],
                            start=(kt == 0),
                            stop=(kt == NKT - 1),
                        )
                nc.vector.tensor_copy(
                    dst[:, nqq * 1024:(nqq + 1) * 1024], ps[:]
                )

    # ---- attention ----
    ctxt = [persist.tile([P, S], BF16, name=f"ctxt{m}") for m in range(4)]

    def attend(hp, qc):
        """Heads (2hp, 2hp+1): even head on partitions 0-63, odd on 64-127."""
        q0 = qc * QC
        hA, hB = 2 * hp, 2 * hp + 1
        ctxA = psum.tile([HD + 1, QC], F32, tag="ctxA", bufs=1)
        ctxB = psum.tile([HD + 1, QC], F32, tag="ctxB", bufs=1)

        def emit_scores(kti):
            psA = psum.tile([P, QC], F32, tag="sc", bufs=2)
            psB = psum.tile([P, QC], F32, tag="sc", bufs=2)
            for half in range(2):
                sl = slice(half * 512, (half + 1) * 512)
                qsl = slice(q0 + half * 512, q0 + (half + 1) * 512)
                # row-group packed pair: A on partitions 0-63, B on 64-127
                nc.tensor.matmul(
                    psA[:, sl],
                    lhsT=kt_sb[hp][0:HD, kti * P:(kti + 1) * P],
                    rhs=qt[hp][0:HD, qsl],
                    start=True, stop=True,
                )
                nc.tensor.matmul(
                    psB[:, sl],
                    lhsT=kt_sb[hp][HD:P, kti * P:(kti + 1) * P],
                    rhs=qt[hp][HD:P, qsl],
                    start=True, stop=True,
                )
            return psA, psB

        def emit_exp(kti, psA, psB):
            ptA = pt_pool.tile([P, QC], BF16, tag="pt", bufs=4)
            ptB = pt_pool.tile([P, QC], BF16, tag="pt", bufs=4)
            nc.scalar.activation(ptA[:], psA[:], Exp, scale=SCALE)
            nc.scalar.activation(ptB[:], psB[:], Exp, scale=SCALE)
            return ptA, ptB

        def emit_ctx(kti, ptA, ptB):
            first = kti == 0
            last = kti == NST - 1
            for half in range(2):
                sl = slice(half * 512, (half + 1) * 512)
                nc.tensor.matmul(
                    ctxA[:, sl], lhsT=vc[kti][:, hA, :], rhs=ptA[:, sl],
                    start=first, stop=last,
                )
                nc.tensor.matmul(
                    ctxB[:, sl], lhsT=vc[kti][:, hB, :], rhs=ptB[:, sl],
                    start=first, stop=last,
                )

        # software pipeline: scores(kt) -> [scores(kt+1) overlaps exp(kt)]
        # -> ctx(kt); keeps ACT streaming while PE stays ahead.
        prev = None
        for kti in range(NST):
            ps = emit_scores(kti)
            if prev is not None:
                emit_ctx(prev[0], prev[1], prev[2])
            ptA, ptB = emit_exp(kti, *ps)
            prev = (kti, ptA, ptB)
        emit_ctx(prev[0], prev[1], prev[2])

        # normalize: ctx^T[d, q] * (1/sum[q]); sums sit in row 64 of ctx'
        for ctxp, rows, h in ((ctxA, slice(0, HD), hA), (ctxB, slice(HD, P), hB)):
            recip = small.tile([1, QC], F32, tag="recip", bufs=4)
            nc.vector.reciprocal(recip[:], ctxp[HD:HD + 1, :])
            bc = psum.tile([HD, QC], F32, tag="sc", bufs=2)
            for half in range(2):
                sl = slice(half * 512, (half + 1) * 512)
                nc.tensor.matmul(
                    bc[:, sl], lhsT=ones_row[:], rhs=recip[:, sl],
                    start=True, stop=True,
                )
            bcs = small.tile([HD, QC], F32, tag="bcs", bufs=2)
            nc.vector.tensor_copy(bcs[:], bc[:])
            nc.vector.tensor_mul(
                ctxt[hp][rows, q0:q0 + QC], ctxp[0:HD, :], bcs[:]
            )

    def outproj(st):
        ps = psum.tile([P, D], F32, tag="sc", bufs=2)
        for half in range(2):
            sl = slice(half * 512, (half + 1) * 512)
            for c in range(4):
                nc.tensor.matmul(
                    ps[:, sl],
                    lhsT=ctxt[c][:, st * P:(st + 1) * P],
                    rhs=wout_bf[c][:, sl],
                    start=(c == 0),
                    stop=(c == 3),
                )
        osb = outsb_pool.tile([P, D], F32, tag="osb", bufs=2)
        nc.scalar.copy(osb[:], ps[:])
        nc.sync.dma_start(out_ap[st * P:(st + 1) * P, :], osb[:])

    project_pair(0)
    for qc in range(2):
        for hp in range(4):
            # emit next pair's projection between attention blocks so the
            # scheduler can fill PE idle time while ACT chews on exps
            attend(hp, qc)
            if qc == 0 and hp < 3:
                project_pair(hp + 1)
        # out-projection for the finished token range overlaps the next phase
        for st in range(qc * (NST // 2), (qc + 1) * (NST // 2)):
            outproj(st)


_CACHED = None


def _get_nc():
    global _CACHED
    if _CACHED is None:
        nc = bacc.Bacc(
            "TRN2", target_bir_lowering=False, debug=False, num_devices=8
        )
        hs = nc.dram_tensor("hs", [S, D], F32, kind="ExternalInput").ap()
        wqkv = nc.dram_tensor("wqkv", [D, 3 * DQ], F32, kind="ExternalInput").ap()
        wout = nc.dram_tensor("wout", [DQ, D], F32, kind="ExternalInput").ap()
        out = nc.dram_tensor("out", [S, D], F32, kind="ExternalOutput").ap()
        build_kernel(nc, out, hs, wqkv, wout)
        nc.compile()
        _CACHED = nc
    return _CACHED


def make_in_maps(hidden_states, w_qkv, w_out):
    in_maps = []
    for c in range(8):
        b, g = divmod(c, 2)
        cols = slice(g * DQ, (g + 1) * DQ)
        wq = w_qkv[:, 0 * D:1 * D][:, cols]
        wk = w_qkv[:, 1 * D:2 * D][:, cols]
        wv = w_qkv[:, 2 * D:3 * D][:, cols]
        in_maps.append({
            "hs": np.ascontiguousarray(hidden_states[b], dtype=np.float32),
            "wqkv": np.ascontiguousarray(
                np.concatenate([wq, wk, wv], axis=1), dtype=np.float32
            ),
            "wout": np.ascontiguousarray(
                w_out[g * DQ:(g + 1) * DQ, :], dtype=np.float32
            ),
        })
    return in_maps


def run(hidden_states, w_qkv, w_out, trace=False):
    nc = _get_nc()
    in_maps = make_in_maps(hidden_states, w_qkv, w_out)
    res = run_bass_kernel_spmd(nc, in_maps, core_ids=list(range(8)), trace=trace)
    out = np.empty((4, S, D), np.float32)
    for b in range(4):
        out[b] = res.results[2 * b]["out"] + res.results[2 * b + 1]["out"]
    return out, res


def kernel(hidden_states, w_qkv, w_out):
    out, _ = run(
        np.asarray(hidden_states), np.asarray(w_qkv), np.asarray(w_out)
    )
    return out


# revision 9
# speedup vs baseline: 1.3045x; 1.2682x over previous
"""AIMv2 attention (B=4, S=2048, D=1024, H=16, d=64) on 8 TRN2 NeuronCores.

Sharding: core c = (batch b = c//2, head-group g = c%2 of 8 heads).
Each core computes its batch's attention for its 8 heads plus the
out-projection partial sum over its heads' rows of w_out; the host adds
the two partials per batch (no on-device collectives needed).

Per-core kernel (all matmuls in bf16, fp32 accumulation):
  X^T via DVE cast + batched XBAR DMA transposes; Q^T,K^T = Wq/k^T @ X^T
  so the score matmuls produce s_T[k, q] directly with heads packed in
  row-groups (even head partitions 0-63, odd 64-127 -> PE row tiling);
  softmax without max-subtraction (scores ~ N(0,1), exp never overflows);
  V carries a ones column so ctx' = [V|1]^T @ P^T yields both ctx^T and
  the softmax denominators in one accumulation; normalization via
  reciprocal_approx_fast + a K=1 broadcast matmul; ctx^T lands in the
  exact lhsT layout the out-projection needs.
"""

import numpy as np

import concourse.bass as bass
import concourse.tile as tile
from concourse import bacc, mybir
from concourse.bass_utils import run_bass_kernel_spmd

P = 128
S = 2048          # sequence length
D = 1024          # model dim
DQ = 512          # per-core qkv width (8 heads x 64)
HD = 64           # head dim
NH = 8            # heads per core
NKT = D // P      # 8 contraction tiles over D
NST = S // P      # 16 tiles over S
QC = 1024         # q chunk for attention inner loop
SCALE = 1.0 / 8.0  # 1/sqrt(64)

F32 = mybir.dt.float32
BF16 = mybir.dt.bfloat16


def build_kernel(nc, out_ap, hs_ap, wqkv_ap, wout_ap):
    import contextlib

    ctx = contextlib.ExitStack()
    with tile.TileContext(nc) as tc:
        with ctx:
            _body(ctx, tc, nc, out_ap, hs_ap, wqkv_ap, wout_ap)


def _body(ctx, tc, nc, out_ap, hs_ap, wqkv_ap, wout_ap):
    Exp = mybir.ActivationFunctionType.Exp

    persist = ctx.enter_context(tc.tile_pool(name="persist", bufs=1))
    psum = ctx.enter_context(tc.tile_pool(name="psum", bufs=1, space="PSUM"))

    # all-ones [128, 64] so a ones-row lhsT can be sliced at any base
    # partition (matmul requires lhsT/rhs base partitions to match)
    ones_rows = persist.tile([P, HD], F32, name="ones_rows")
    nc.vector.memset(ones_rows[:], 1.0)

    # ---- load phase (staging pools released before attention pools open) --
    # X^T lives as one 3D tile so each X row-tile transposes in ONE XBAR DMA.
    xt3 = persist.tile([P, NKT, S], BF16, name="xt3")
    wqkv_bf = []
    wout_bf = []
    with tc.tile_pool(name="stage", bufs=3) as stage:
        for kt in range(NKT):
            wf = stage.tile([P, 3 * DQ], F32, tag="wstage", bufs=2)
            nc.sync.dma_start(wf[:], wqkv_ap[kt * P:(kt + 1) * P, :])
            wb = persist.tile([P, 3 * DQ], BF16, name=f"wqkv_bf{kt}")
            nc.scalar.copy(wb[:], wf[:])
            wqkv_bf.append(wb)

        for i in range(DQ // P):
            wf = stage.tile([P, D], F32, tag="wostage", bufs=2)
            nc.sync.dma_start(wf[:], wout_ap[i * P:(i + 1) * P, :])
            wb = persist.tile([P, D], BF16, name=f"wout_bf{i}")
            nc.scalar.copy(wb[:], wf[:])
            wout_bf.append(wb)

        # X: load, cast bf16, batched XBAR transpose into [D, S] layout
        for st in range(NST):
            xf = stage.tile([P, D], F32, tag="xstage", bufs=3)
            nc.sync.dma_start(xf[:], hs_ap[st * P:(st + 1) * P, :])
            xb = stage.tile([P, D], BF16, tag="xbf", bufs=3)
            nc.vector.tensor_copy(xb[:], xf[:])
            nc.sync.dma_start_transpose(xt3[:, :, st * P:(st + 1) * P], xb[:])

    pt_pool = ctx.enter_context(tc.tile_pool(name="pt", bufs=4))
    small = ctx.enter_context(tc.tile_pool(name="small", bufs=4))
    outsb_pool = ctx.enter_context(tc.tile_pool(name="outsb", bufs=2))

    def xt(kt):
        return xt3[:, kt, :]

    # ---- V projection with ones column: vc[st][:, h, 0:64]=V_h, [...,64]=1
    vc = [persist.tile([P, NH, HD + 1], BF16, name=f"vc{st}") for st in range(NST)]
    for st in range(NST):
        nc.vector.memset(vc[st][:, :, HD:HD + 1], 1.0)
    for stq in range(NST // 2):
        ps = psum.tile([P, 2 * DQ], F32, tag="sc", bufs=2)
        for half in range(2):
            st = 2 * stq + half
            sl = slice(half * DQ, (half + 1) * DQ)
            for kt in range(NKT):
                nc.tensor.matmul(
                    ps[:, sl],
                    lhsT=xt(kt)[:, st * P:(st + 1) * P],
                    rhs=wqkv_bf[kt][:, 2 * DQ:3 * DQ],
                    start=(kt == 0),
                    stop=(kt == NKT - 1),
                )
        for half in range(2):
            st = 2 * stq + half
            for h in range(NH):
                nc.scalar.copy(
                    vc[st][:, h, 0:HD],
                    ps[:, half * DQ + h * HD: half * DQ + (h + 1) * HD],
                )

    # ---- Q^T / K^T projections, per head pair ----
    qt = [persist.tile([P, S], BF16, name=f"qt{m}") for m in range(4)]
    kt_sb = [persist.tile([P, S], BF16, name=f"kt{m}") for m in range(4)]

    def project_pair(m):
        # rows 128m..128m+128 of Q^T and K^T (heads 2m, 2m+1)
        for which, dst in ((0, qt[m]), (DQ, kt_sb[m])):
            for nqq in range(2):
                ps = psum.tile([P, 2 * 512], F32, tag="sc", bufs=2)
                for half in range(2):
                    nq = 2 * nqq + half
                    sl = slice(half * 512, (half + 1) * 512)
                    for kt in range(NKT):
                        nc.tensor.matmul(
                            ps[:, sl],
                            lhsT=wqkv_bf[kt][:, which + m * P: which + (m + 1) * P],
                            rhs=xt(kt)[:, nq * 512:(nq + 1) * 512],
                            start=(kt == 0),
                            stop=(kt == NKT - 1),
                        )
                nc.vector.tensor_copy(
                    dst[:, nqq * 1024:(nqq + 1) * 1024], ps[:]
                )

    # ---- attention ----
    ctxt = [persist.tile([P, S], BF16, name=f"ctxt{m}") for m in range(4)]

    def attend(hp, qc):
        """Heads (2hp, 2hp+1): even head on partitions 0-63, odd on 64-127."""
        q0 = qc * QC
        hA, hB = 2 * hp, 2 * hp + 1
        ctxA = psum.tile([HD + 1, QC], F32, tag="ctxA", bufs=1)
        ctxB = psum.tile([HD + 1, QC], F32, tag="ctxB", bufs=1)

        def emit_scores(kti):
            psA = psum.tile([P, QC], F32, tag="sc", bufs=2)
            psB = psum.tile([P, QC], F32, tag="sc", bufs=2)
            for half in range(2):
                sl = slice(half * 512, (half + 1) * 512)
                qsl = slice(q0 + half * 512, q0 + (half + 1) * 512)
                # row-group packed pair: A on partitions 0-63, B on 64-127
                nc.tensor.matmul(
                    psA[:, sl],
                    lhsT=kt_sb[hp][0:HD, kti * P:(kti + 1) * P],
                    rhs=qt[hp][0:HD, qsl],
                    start=True, stop=True,
                )
                nc.tensor.matmul(
                    psB[:, sl],
                    lhsT=kt_sb[hp][HD:P, kti * P:(kti + 1) * P],
                    rhs=qt[hp][HD:P, qsl],
                    start=True, stop=True,
                )
            return psA, psB

        def emit_exp(kti, psA, psB):
            ptA = pt_pool.tile([P, QC], BF16, tag="pt", bufs=4)
            ptB = pt_pool.tile([P, QC], BF16, tag="pt", bufs=4)
            nc.scalar.activation(ptA[:], psA[:], Exp, scale=SCALE)
            nc.scalar.activation(ptB[:], psB[:], Exp, scale=SCALE)
            return ptA, ptB

        def emit_ctx(kti, ptA, ptB):
            first = kti == 0
            last = kti == NST - 1
            for half in range(2):
                sl = slice(half * 512, (half + 1) * 512)
                nc.tensor.matmul(
                    ctxA[:, sl], lhsT=vc[kti][:, hA, :], rhs=ptA[:, sl],
                    start=first, stop=last,
                )
                nc.tensor.matmul(
                    ctxB[:, sl], lhsT=vc[kti][:, hB, :], rhs=ptB[:, sl],
                    start=first, stop=last,
                )

        # software pipeline: scores(kt) -> [scores(kt+1) overlaps exp(kt)]
        # -> ctx(kt); keeps ACT streaming while PE stays ahead.
        prev = None
        for kti in range(NST):
            ps = emit_scores(kti)
            if prev is not None:
                emit_ctx(prev[0], prev[1], prev[2])
            ptA, ptB = emit_exp(kti, *ps)
            prev = (kti, ptA, ptB)
        emit_ctx(prev[0], prev[1], prev[2])

        # normalize: ctx^T[d, q] * (1/sum[q]); sums sit in row 64 of ctx'.
        # Copy ctx' to SBUF first so the PSUM accumulators free immediately,
        # broadcast the raw sums by matmul, then approx-reciprocal the
        # 64-partition block (the approx op misbehaves on base_partition 64).
        for ctxp, rows, h in ((ctxA, slice(0, HD), hA), (ctxB, slice(HD, P), hB)):
            csb = small.tile([HD + 1, QC], F32, tag="csb", bufs=4)
            nc.vector.tensor_copy(csb[:], ctxp[:])
            bc = psum.tile([HD, QC], F32, tag="sc", bufs=2)
            for half in range(2):
                sl = slice(half * 512, (half + 1) * 512)
                nc.tensor.matmul(
                    bc[:, sl], lhsT=ones_rows[HD:HD + 1, :],
                    rhs=csb[HD:HD + 1, sl],
                    start=True, stop=True,
                )
            rec = small.tile([HD, QC], F32, tag="rec", bufs=2)
            nc.vector.reciprocal_approx_fast(rec[:], bc[:])
            nc.vector.tensor_mul(
                ctxt[hp][rows, q0:q0 + QC], csb[0:HD, :], rec[:]
            )

    def outproj(st):
        ps = psum.tile([P, D], F32, tag="sc", bufs=2)
        for half in range(2):
            sl = slice(half * 512, (half + 1) * 512)
            for c in range(4):
                nc.tensor.matmul(
                    ps[:, sl],
                    lhsT=ctxt[c][:, st * P:(st + 1) * P],
                    rhs=wout_bf[c][:, sl],
                    start=(c == 0),
                    stop=(c == 3),
                )
        osb = outsb_pool.tile([P, D], F32, tag="osb", bufs=2)
        nc.scalar.copy(osb[:], ps[:])
        nc.sync.dma_start(out_ap[st * P:(st + 1) * P, :], osb[:])

    project_pair(0)
    for qc in range(2):
        for hp in range(4):
            # emit next pair's projection between attention blocks so the
            # scheduler can fill PE idle time while ACT chews on exps
            attend(hp, qc)
            if qc == 0 and hp < 3:
                project_pair(hp + 1)
        # out-projection for the finished token range overlaps the next phase
        for st in range(qc * (NST // 2), (qc + 1) * (NST // 2)):
            outproj(st)


_CACHED = None


def _get_nc():
    global _CACHED
    if _CACHED is None:
        nc = bacc.Bacc(
            "TRN2", target_bir_lowering=False, debug=False, num_devices=8
        )
        hs = nc.dram_tensor("hs", [S, D], F32, kind="ExternalInput").ap()
        wqkv = nc.dram_tensor("wqkv", [D, 3 * DQ], F32, kind="ExternalInput").ap()
        wout = nc.dram_tensor("wout", [DQ, D], F32, kind="ExternalInput").ap()
        out = nc.dram_tensor("out", [S, D], F32, kind="ExternalOutput").ap()
        build_kernel(nc, out, hs, wqkv, wout)
        nc.compile()
        _CACHED = nc
    return _CACHED


def make_in_maps(hidden_states, w_qkv, w_out):
    in_maps = []
    for c in range(8):
        b, g = divmod(c, 2)
        cols = slice(g * DQ, (g + 1) * DQ)
        wq = w_qkv[:, 0 * D:1 * D][:, cols]
        wk = w_qkv[:, 1 * D:2 * D][:, cols]
        wv = w_qkv[:, 2 * D:3 * D][:, cols]
        in_maps.append({
            "hs": np.ascontiguousarray(hidden_states[b], dtype=np.float32),
            "wqkv": np.ascontiguousarray(
                np.concatenate([wq, wk, wv], axis=1), dtype=np.float32
            ),
            "wout": np.ascontiguousarray(
                w_out[g * DQ:(g + 1) * DQ, :], dtype=np.float32
            ),
        })
    return in_maps


def run(hidden_states, w_qkv, w_out, trace=False):
    nc = _get_nc()
    in_maps = make_in_maps(hidden_states, w_qkv, w_out)
    res = run_bass_kernel_spmd(nc, in_maps, core_ids=list(range(8)), trace=trace)
    out = np.empty((4, S, D), np.float32)
    for b in range(4):
        out[b] = res.results[2 * b]["out"] + res.results[2 * b + 1]["out"]
    return out, res


def kernel(hidden_states, w_qkv, w_out):
    out, _ = run(
        np.asarray(hidden_states), np.asarray(w_qkv), np.asarray(w_out)
    )
    return out
